# revision 22
# baseline (speedup 1.0000x reference)
"""Trainium2 Bass kernel for nn_MemoryAugmented (scatter_memory).

Computes, for full inputs x:[64,12,883,64], M:[12,64,64]:
    score = softmax(einsum('blnd,tmd->btnm', x, M), axis=-1)
    out   = einsum('btnm,tmd->btnd', score, M)

Distribution: data-parallel over batch across 8 NeuronCores (8 batches
per core); the small memory bank M is replicated.

The device pipeline runs in 16-bit (fp32 PSUM): fp16 on the input side
(x, l-sum tree, mm1 weights, final output) -- fp16 matmuls run at full
PE rate, the DMA bytes halve vs fp32 (the binding resource: ~11 MB in +
~10.5 MB out per core at ~358 GB/s), and fp16's 2^-11 rounding keeps
end-to-end error ~3e-3. Only exp's output uses bf16 (e^~19 overflows
fp16) and the softmax reciprocal stays fp32 (1/sum underflows fp16).

Engine assignment per 512-row tile (14 per core), set by trace data:
  DVE     4x (strided norm-multiply ~0.95us + reciprocal ~0.22us) -- the
          PSUM-evacuation floor (PSUM is readable only by DVE/ACT and
          f32 PSUM reads run at 1 elem/cycle) -- plus the last tree add
  gpsimd  the two big l-sum tree adds (slow engine, but otherwise idle)
  ACT     3x Exp [128,1024] (merged logit pairs)
  PE      6 mm1 (N=512) + 24 mm2 (N=130) + LDWEIGHTS (fp16 full rate)
  sync    loads + stores (both fully contiguous per partition)

Per-core dataflow (rows r = (b, n) flattened to 7064, padded to 14*512):
  host     x -> fp16, laid out [tile, p=(l_half, d), l%6, r]
  load     one 768 KB DMA per tile (sync ring, 6 KB runs/part)
  tree     l-sum 6->3->2 on gpsimd, ->1 on DVE (fp16, 2x mode); the
           final l_half sum folds into mm1's K=128 contraction
           (weights replicated across both halves)
  mm1      6x matmul(mwT_pair fp16 [128,128], xs [128,512]) -> logits
  exp      3x ACT Exp [128,1024] PSUM -> SBUF bf16 (|logits| < ~19)
  mm2      per 128-row chunk: exp_chunk^T @ [blockdiag(M) | ones cols]
           -> [rows, (t0 d | t1 d | sums)] PSUM; DVE strided reciprocal
           of the 12 sums + broadcast multiply normalizes into vn fp16
  store    one 768 KB DMA per tile (sync ring, 6 KB runs/part)
"""
import sys

for _p in ("/opt/trn_rl_repo",):
    if _p not in sys.path:
        sys.path.insert(0, _p)

from contextlib import ExitStack

import numpy as np

import concourse.bass as bass
import concourse.bacc as bacc
import concourse.tile as tile
from concourse import mybir
from concourse._compat import with_exitstack
from concourse.bass_utils import run_bass_kernel_spmd

B, L, N, D = 64, 12, 883, 64
T, MNUM = 12, 64
NCORES = 8
BS = B // NCORES          # 8 batches per core
ROWS = BS * N             # 7064 real rows per core
NTILES = 14               # 14 tiles of 512 rows (7168, zero-padded)
TR = 512                  # rows per tile
NCH = TR // 128           # 8 chunks per tile
RP = NTILES * TR
F32 = mybir.dt.float32
BF16 = mybir.dt.bfloat16
FP16 = mybir.dt.float16
F16 = np.float16


def build_consts(M):
    """Host-side layout prep (pure data movement) of the memory bank."""
    M = np.asarray(M, dtype=np.float32)
    mw = np.zeros((128, 6 * 128), np.float32)
    mbd = np.zeros((128, 6 * 130), np.float32)
    for tp in range(6):
        t0, t1 = 2 * tp, 2 * tp + 1
        for lh in range(2):
            mw[lh * 64:(lh + 1) * 64, tp * 128 + 0:tp * 128 + 64] = M[t0].T
            mw[lh * 64:(lh + 1) * 64, tp * 128 + 64:tp * 128 + 128] = M[t1].T
        mbd[0:64, tp * 130 + 0:tp * 130 + 64] = M[t0]
        mbd[64:128, tp * 130 + 64:tp * 130 + 128] = M[t1]
        mbd[0:64, tp * 130 + 128] = 1.0
        mbd[64:128, tp * 130 + 129] = 1.0
    return mw.astype(F16), mbd.astype(F16)


@with_exitstack
def kernel_body(ctx: ExitStack, tc: "tile.TileContext", out: bass.AP,
                x: bass.AP, mw: bass.AP, mbd: bass.AP):
    nc = tc.nc
    consts = ctx.enter_context(tc.tile_pool(name="consts", bufs=1))
    work = ctx.enter_context(tc.tile_pool(name="work", bufs=2))
    psum = ctx.enter_context(tc.tile_pool(name="psum", bufs=1, space="PSUM"))

    mw_sb = consts.tile([128, 6 * 128], FP16)
    nc.scalar.dma_start(out=mw_sb[:], in_=mw[:])
    mbd_sb = consts.tile([128, 6 * 130], FP16)
    nc.scalar.dma_start(out=mbd_sb[:], in_=mbd[:])
    zbias = consts.tile([128, 1], F32)
    nc.vector.memset(zbias[:], 0.0)
    # warm the ACT exp table set (~2.7us table load) off the critical path
    warm = consts.tile([128, 1], F32)
    nc.scalar.activation(warm[:], zbias[:],
                         mybir.ActivationFunctionType.Exp, bias=zbias[:])

    # software-pipelined loads: issue each tile's load 2 iterations ahead
    # so the sync queue never gates the gpsimd tree on a late prefetch
    xts = {}

    def prefetch(tj):
        if tj < NTILES:
            xt_new = work.tile([128, 6 * TR], FP16, tag="xt", bufs=4)
            nc.sync.dma_start(out=xt_new[:], in_=x[tj])
            xts[tj] = xt_new

    # ---- ramp-in: tile 0 as two 256-row mini-tiles to halve the
    # pipeline-fill latency (a full tile takes ~20us to traverse) ----
    for r0 in (0, 256):
        xm = work.tile([128, 6 * 256], FP16, tag="xm", bufs=2)
        xmv = xm[:].rearrange("p (s r) -> p s r", s=6)
        nc.sync.dma_start(
            out=xmv,
            in_=x[0].rearrange("p (s r) -> p s r", s=6)[:, :, r0:r0 + 256])
        xvm = xm[:].rearrange("p (l two r) -> p l two r", two=2, r=256)
        t3m = work.tile([128, 3 * 256], FP16, tag="t3m", bufs=2)
        t3mv = t3m[:].rearrange("p (l r) -> p l r", l=3)
        nc.gpsimd.tensor_add(t3mv, xvm[:, :, 0], xvm[:, :, 1])
        t2m = work.tile([128, 256], FP16, tag="t2m", bufs=2)
        nc.gpsimd.tensor_add(t2m[:], t3mv[:, 0], t3mv[:, 1])
        xsm = work.tile([128, 256], FP16, tag="xsm", bufs=2)
        nc.vector.tensor_add(xsm[:], t2m[:], t3mv[:, 2])
        mexps = []
        for pi in range(3):
            ps_log = psum.tile([128, 1024], F32, tag="logits", bufs=2)
            for half in range(2):
                tp = 2 * pi + half
                nc.tensor.matmul(ps_log[:, half * 256:(half + 1) * 256],
                                 mw_sb[:, tp * 128:(tp + 1) * 128],
                                 xsm[:], start=True, stop=True)
            ex = work.tile([128, 1024], BF16, tag="exp", bufs=8)
            nc.scalar.activation(ex[:, 0:512], ps_log[:, 0:512],
                                 mybir.ActivationFunctionType.Exp, bias=zbias[:])
            mexps.append(ex)
        vnm = work.tile([128, NCH * T * D], FP16, tag="vn", bufs=3)
        for c in range(2):
            ps_val = psum.tile([128, 1024], F32, tag="val", bufs=2)
            for tp in range(6):
                off = 512 * (tp // 3) + 130 * (tp % 3)
                nc.tensor.matmul(
                    ps_val[:, off:off + 130],
                    mexps[tp // 2][:, (tp % 2) * 256 + c * 128:
                                   (tp % 2) * 256 + (c + 1) * 128],
                    mbd_sb[:, tp * 130:(tp + 1) * 130],
                    start=True, stop=True)
            sums_ap = (ps_val[:].rearrange("p (h r) -> p h r", h=2)
                       [:, :, 0:390]
                       .rearrange("p h (a r) -> p h a r", a=3)
                       [:, :, :, 128:130])
            recm = work.tile([128, 12], F32, tag="rec", bufs=6)
            nc.vector.reciprocal(
                recm[:].rearrange("p (h a t) -> p h a t", h=2, a=3), sums_ap)
            in0 = (ps_val[:].rearrange("p (h r) -> p h r", h=2)
                   [:, :, 0:390]
                   .rearrange("p h (a r) -> p h a r", a=3)
                   [:, :, :, 0:128]
                   .rearrange("p h a (t d) -> p h a t d", t=2))
            in1 = (recm[:].rearrange("p (h a t) -> p h a t", h=2, a=3)
                   .unsqueeze(4)
                   .broadcast_to([128, 2, 3, 2, D]))
            outp = (vnm[:, c * 768:(c + 1) * 768]
                    .rearrange("p (h a t d) -> p h a t d", h=2, a=3, t=2))
            nc.vector.tensor_mul(outp, in0, in1)
        nc.scalar.dma_start(
            out=out[0][:, (r0 // 128) * 768:(r0 // 128 + 2) * 768],
            in_=vnm[:, 0:2 * 768])

    prefetch(1)
    prefetch(2)

    for ti in range(1, NTILES):
        prefetch(ti + 2)
        # ---- l-sum tree (6 slabs -> 1) on gpsimd, fp16 ----
        xt = xts.pop(ti)
        xv = xt[:].rearrange("p (l two r) -> p l two r", two=2, r=TR)
        t3 = work.tile([128, 3 * TR], FP16, tag="t3", bufs=3)
        t3v = t3[:].rearrange("p (l r) -> p l r", l=3)
        nc.gpsimd.tensor_add(t3v, xv[:, :, 0], xv[:, :, 1])
        t2 = work.tile([128, TR], FP16, tag="t2", bufs=2)
        nc.gpsimd.tensor_add(t2[:], t3v[:, 0], t3v[:, 1])
        xs = work.tile([128, TR], FP16, tag="xs", bufs=3)
        # alternate the last tree add: DVE is the pacer, gpsimd has slack
        eng = nc.vector if ti % 2 == 0 else nc.gpsimd
        eng.tensor_add(xs[:], t2[:], t3v[:, 2])

        # ---- mm1 + exp ----
        exps = []
        for pi in range(3):
            ps_log = psum.tile([128, 1024], F32, tag="logits", bufs=2)
            for half in range(2):
                tp = 2 * pi + half
                nc.tensor.matmul(ps_log[:, half * 512:(half + 1) * 512],
                                 mw_sb[:, tp * 128:(tp + 1) * 128],
                                 xs[:], start=True, stop=True)
            ex = work.tile([128, 1024], BF16, tag="exp", bufs=8)
            nc.scalar.activation(ex[:], ps_log[:],
                                 mybir.ActivationFunctionType.Exp, bias=zbias[:])
            exps.append(ex)

        def expv(tp):
            return exps[tp // 2][:, (tp % 2) * 512:(tp % 2 + 1) * 512]

        # ---- mm2 + normalize per 128-row chunk ----
        vn = work.tile([128, NCH * T * D], FP16, tag="vn", bufs=3)
        for c in range(NCH):
            ps_val = psum.tile([128, 1024], F32, tag="val", bufs=2)
            for tp in range(6):
                off = 512 * (tp // 3) + 130 * (tp % 3)
                nc.tensor.matmul(ps_val[:, off:off + 130],
                                 expv(tp)[:, c * 128:(c + 1) * 128],
                                 mbd_sb[:, tp * 130:(tp + 1) * 130],
                                 start=True, stop=True)
            sums_ap = (ps_val[:].rearrange("p (h r) -> p h r", h=2)
                       [:, :, 0:390]
                       .rearrange("p h (a r) -> p h a r", a=3)
                       [:, :, :, 128:130])
            rec = work.tile([128, 12], F32, tag="rec", bufs=6)
            nc.vector.reciprocal(
                rec[:].rearrange("p (h a t) -> p h a t", h=2, a=3), sums_ap)
            in0 = (ps_val[:].rearrange("p (h r) -> p h r", h=2)
                   [:, :, 0:390]
                   .rearrange("p h (a r) -> p h a r", a=3)
                   [:, :, :, 0:128]
                   .rearrange("p h a (t d) -> p h a t d", t=2))
            in1 = (rec[:].rearrange("p (h a t) -> p h a t", h=2, a=3)
                   .unsqueeze(4)
                   .broadcast_to([128, 2, 3, 2, D]))
            outp = (vn[:, c * 768:(c + 1) * 768]
                    .rearrange("p (h a t d) -> p h a t d", h=2, a=3, t=2))
            nc.vector.tensor_mul(outp, in0, in1)
        # one fully-contiguous store per tile on the ACT HWDGE ring (keeps
        # the sync ring loads-only so prefetches issue unboundedly ahead);
        # 128 descriptors of 6 KB; host unshuffles [ti, p, c, t*d]
        nc.scalar.dma_start(out=out[ti], in_=vn[:])


_NC_CACHE = {}


def build_nc():
    if "nc" in _NC_CACHE:
        return _NC_CACHE["nc"]
    nc = bacc.Bacc("TRN2", target_bir_lowering=False, debug=False,
                   num_devices=NCORES)
    x_ap = nc.dram_tensor("x_sh", [NTILES, 128, 6 * TR], FP16,
                          kind="ExternalInput").ap()
    mw_ap = nc.dram_tensor("mw", [128, 6 * 128], FP16, kind="ExternalInput").ap()
    mbd_ap = nc.dram_tensor("mbd", [128, 6 * 130], FP16, kind="ExternalInput").ap()
    out_ap = nc.dram_tensor("out", [NTILES, 128, NCH * T * D], FP16,
                            kind="ExternalOutput").ap()
    with tile.TileContext(nc) as tc:
        kernel_body(tc, out_ap, x_ap, mw_ap, mbd_ap)
    nc.compile()
    _NC_CACHE["nc"] = nc
    return nc


def make_in_maps(x, M):
    xf = np.asarray(x).astype(F16)
    mw, mbd = build_consts(M)
    maps = []
    for i in range(NCORES):
        xc = xf[i * BS:(i + 1) * BS]                     # (8, 12, 883, 64)
        xc = xc.reshape(BS, 2, 6, N, D)                  # (b, lh, lr, n, d)
        xc = xc.transpose(0, 3, 1, 4, 2)                 # (b, n, lh, d, lr)
        xc = xc.reshape(ROWS, 2, D, 6)
        xp = np.zeros((RP, 2, D, 6), F16)
        xp[:ROWS] = xc
        xp = (xp.reshape(NTILES, TR, 128, 6)
                .transpose(0, 2, 3, 1)                   # (ti, p, lr, r)
                .reshape(NTILES, 128, 6 * TR))
        maps.append({"x_sh": np.ascontiguousarray(xp), "mw": mw, "mbd": mbd})
    return maps


def unshard_out(res):
    outs = []
    for i in range(NCORES):
        o = np.asarray(res[i]["out"]).astype(np.float32)   # [ti, p, c*768]
        o = (o.reshape(NTILES, 128, NCH, T * D)
              .transpose(0, 2, 1, 3)                       # row = ti*TR+c*128+p
              .reshape(RP, T * D))[:ROWS]
        outs.append(o.reshape(BS, N, T, D).transpose(0, 2, 1, 3))
    return np.ascontiguousarray(np.concatenate(outs, axis=0))


def kernel(x, M):
    nc = build_nc()
    in_maps = make_in_maps(x, M)
    res = run_bass_kernel_spmd(nc, in_maps, list(range(NCORES))).results
    return unshard_out(res)


if __name__ == "__main__":
    rng = np.random.default_rng(0)
    x = rng.standard_normal((B, L, N, D), dtype=np.float32)
    M = (rng.standard_normal((T, MNUM, D), dtype=np.float32) * 0.125).astype(np.float32)
    out = kernel(x, M)
    print("out", out.shape, out.dtype, float(np.abs(out).max()))


# revision 27
# speedup vs baseline: 1.0168x; 1.0168x over previous
"""Trainium2 Bass kernel for nn_MemoryAugmented (scatter_memory).

Computes, for full inputs x:[64,12,883,64], M:[12,64,64]:
    score = softmax(einsum('blnd,tmd->btnm', x, M), axis=-1)
    out   = einsum('btnm,tmd->btnd', score, M)

Distribution: data-parallel over batch across 8 NeuronCores (8 batches
per core); the small memory bank M is replicated.

The device pipeline runs in 16-bit (fp32 PSUM): fp16 on the input side
(x, l-sum tree, mm1 weights, final output) -- fp16 matmuls run at full
PE rate, the DMA bytes halve vs fp32 (the binding resource: ~11 MB in +
~10.5 MB out per core at ~358 GB/s), and fp16's 2^-11 rounding keeps
end-to-end error ~3e-3. Only exp's output uses bf16 (e^~19 overflows
fp16) and the softmax reciprocal stays fp32 (1/sum underflows fp16).

Engine assignment per 512-row tile (14 per core), set by trace data:
  DVE     4x (strided norm-multiply ~0.95us + reciprocal ~0.22us) -- the
          PSUM-evacuation floor (PSUM is readable only by DVE/ACT and
          f32 PSUM reads run at 1 elem/cycle) -- plus the last tree add
  gpsimd  the two big l-sum tree adds (slow engine, but otherwise idle)
  ACT     3x Exp [128,1024] (merged logit pairs)
  PE      6 mm1 (N=512) + 24 mm2 (N=130) + LDWEIGHTS (fp16 full rate)
  sync    loads + stores (both fully contiguous per partition)

Per-core dataflow (rows r = (b, n) flattened to 7064, padded to 14*512):
  host     x -> fp16, laid out [tile, p=(l_half, d), l%6, r]
  load     one 768 KB DMA per tile (sync ring, 6 KB runs/part)
  tree     l-sum 6->3->2 on gpsimd, ->1 on DVE (fp16, 2x mode); the
           final l_half sum folds into mm1's K=128 contraction
           (weights replicated across both halves)
  mm1      6x matmul(mwT_pair fp16 [128,128], xs [128,512]) -> logits
  exp      3x ACT Exp [128,1024] PSUM -> SBUF bf16 (|logits| < ~19)
  mm2      per 128-row chunk: exp_chunk^T @ [blockdiag(M) | ones cols]
           -> [rows, (t0 d | t1 d | sums)] PSUM; DVE strided reciprocal
           of the 12 sums + broadcast multiply normalizes into vn fp16
  store    one 768 KB DMA per tile (sync ring, 6 KB runs/part)
"""
import sys

for _p in ("/opt/trn_rl_repo",):
    if _p not in sys.path:
        sys.path.insert(0, _p)

from contextlib import ExitStack

import numpy as np

import concourse.bass as bass
import concourse.bacc as bacc
import concourse.tile as tile
from concourse import mybir
from concourse._compat import with_exitstack
from concourse.bass_utils import run_bass_kernel_spmd

B, L, N, D = 64, 12, 883, 64
T, MNUM = 12, 64
NCORES = 8
BS = B // NCORES          # 8 batches per core
ROWS = BS * N             # 7064 real rows per core
NTILES = 14               # 14 tiles of 512 rows (7168, zero-padded)
TR = 512                  # rows per tile
NCH = TR // 128           # 8 chunks per tile
RP = NTILES * TR
F32 = mybir.dt.float32
BF16 = mybir.dt.bfloat16
FP16 = mybir.dt.float16
F16 = np.float16


def build_consts(M):
    """Host-side layout prep (pure data movement) of the memory bank."""
    M = np.asarray(M, dtype=np.float32)
    mw = np.zeros((128, 6 * 128), np.float32)
    mbd = np.zeros((128, 6 * 130), np.float32)
    for tp in range(6):
        t0, t1 = 2 * tp, 2 * tp + 1
        for lh in range(2):
            mw[lh * 64:(lh + 1) * 64, tp * 128 + 0:tp * 128 + 64] = M[t0].T
            mw[lh * 64:(lh + 1) * 64, tp * 128 + 64:tp * 128 + 128] = M[t1].T
        mbd[0:64, tp * 130 + 0:tp * 130 + 64] = M[t0]
        mbd[64:128, tp * 130 + 64:tp * 130 + 128] = M[t1]
        mbd[0:64, tp * 130 + 128] = 1.0
        mbd[64:128, tp * 130 + 129] = 1.0
    return mw.astype(F16), mbd.astype(F16)


@with_exitstack
def kernel_body(ctx: ExitStack, tc: "tile.TileContext", out: bass.AP,
                x: bass.AP, mw: bass.AP, mbd: bass.AP):
    nc = tc.nc
    consts = ctx.enter_context(tc.tile_pool(name="consts", bufs=1))
    work = ctx.enter_context(tc.tile_pool(name="work", bufs=2))
    psum = ctx.enter_context(tc.tile_pool(name="psum", bufs=1, space="PSUM"))

    mw_sb = consts.tile([128, 6 * 128], FP16)
    nc.scalar.dma_start(out=mw_sb[:], in_=mw[:])
    mbd_sb = consts.tile([128, 6 * 130], FP16)
    nc.scalar.dma_start(out=mbd_sb[:], in_=mbd[:])
    zbias = consts.tile([128, 1], F32)
    nc.vector.memset(zbias[:], 0.0)
    # warm the ACT exp table set (~2.7us table load) off the critical path
    warm = consts.tile([128, 1], F32)
    nc.scalar.activation(warm[:], zbias[:],
                         mybir.ActivationFunctionType.Exp, bias=zbias[:])

    # software-pipelined loads: issue each tile's load 2 iterations ahead
    # so the sync queue never gates the gpsimd tree on a late prefetch
    xts = {}

    def prefetch(tj):
        if tj < NTILES:
            xt_new = work.tile([128, 6 * TR], FP16, tag="xt", bufs=4)
            nc.sync.dma_start(out=xt_new[:], in_=x[tj])
            xts[tj] = xt_new

    # ---- ramp-in: tiles 0-1 as 256-row mini-tiles to shorten the
    # pipeline-fill latency (a cold full tile takes ~15-20us to traverse;
    # the steady cadence of 5.2us/tile is only reached ~3 tiles in) ----
    for mti, r0 in ((0, 0), (0, 256), (1, 0), (1, 256)):
        xm = work.tile([128, 6 * 256], FP16, tag="xm", bufs=2)
        xmv = xm[:].rearrange("p (s r) -> p s r", s=6)
        nc.sync.dma_start(
            out=xmv,
            in_=x[mti].rearrange("p (s r) -> p s r", s=6)[:, :, r0:r0 + 256])
        xvm = xm[:].rearrange("p (l two r) -> p l two r", two=2, r=256)
        t3m = work.tile([128, 3 * 256], FP16, tag="t3m", bufs=2)
        t3mv = t3m[:].rearrange("p (l r) -> p l r", l=3)
        nc.gpsimd.tensor_add(t3mv, xvm[:, :, 0], xvm[:, :, 1])
        t2m = work.tile([128, 256], FP16, tag="t2m", bufs=2)
        nc.gpsimd.tensor_add(t2m[:], t3mv[:, 0], t3mv[:, 1])
        xsm = work.tile([128, 256], FP16, tag="xsm", bufs=2)
        nc.vector.tensor_add(xsm[:], t2m[:], t3mv[:, 2])
        mexps = []
        for pi in range(3):
            ps_log = psum.tile([128, 1024], F32, tag="logits", bufs=2)
            for half in range(2):
                tp = 2 * pi + half
                nc.tensor.matmul(ps_log[:, half * 256:(half + 1) * 256],
                                 mw_sb[:, tp * 128:(tp + 1) * 128],
                                 xsm[:], start=True, stop=True)
            ex = work.tile([128, 1024], BF16, tag="exp", bufs=8)
            nc.scalar.activation(ex[:, 0:512], ps_log[:, 0:512],
                                 mybir.ActivationFunctionType.Exp, bias=zbias[:])
            mexps.append(ex)
        vnm = work.tile([128, NCH * T * D], FP16, tag="vn", bufs=3)
        for c in range(2):
            ps_val = psum.tile([128, 1024], F32, tag="val", bufs=2)
            for tp in range(6):
                off = 512 * (tp // 3) + 130 * (tp % 3)
                nc.tensor.matmul(
                    ps_val[:, off:off + 130],
                    mexps[tp // 2][:, (tp % 2) * 256 + c * 128:
                                   (tp % 2) * 256 + (c + 1) * 128],
                    mbd_sb[:, tp * 130:(tp + 1) * 130],
                    start=True, stop=True)
            sums_ap = (ps_val[:].rearrange("p (h r) -> p h r", h=2)
                       [:, :, 0:390]
                       .rearrange("p h (a r) -> p h a r", a=3)
                       [:, :, :, 128:130])
            recm = work.tile([128, 12], F32, tag="rec", bufs=6)
            nc.vector.reciprocal(
                recm[:].rearrange("p (h a t) -> p h a t", h=2, a=3), sums_ap)
            in0 = (ps_val[:].rearrange("p (h r) -> p h r", h=2)
                   [:, :, 0:390]
                   .rearrange("p h (a r) -> p h a r", a=3)
                   [:, :, :, 0:128]
                   .rearrange("p h a (t d) -> p h a t d", t=2))
            in1 = (recm[:].rearrange("p (h a t) -> p h a t", h=2, a=3)
                   .unsqueeze(4)
                   .broadcast_to([128, 2, 3, 2, D]))
            outp = (vnm[:, c * 768:(c + 1) * 768]
                    .rearrange("p (h a t d) -> p h a t d", h=2, a=3, t=2))
            nc.vector.tensor_mul(outp, in0, in1)
        nc.scalar.dma_start(
            out=out[mti][:, (r0 // 128) * 768:(r0 // 128 + 2) * 768],
            in_=vnm[:, 0:2 * 768])

    prefetch(2)
    prefetch(3)

    for ti in range(2, NTILES):
        prefetch(ti + 2)
        # ---- l-sum tree (6 slabs -> 1) on gpsimd, fp16 ----
        xt = xts.pop(ti)
        xv = xt[:].rearrange("p (l two r) -> p l two r", two=2, r=TR)
        t3 = work.tile([128, 3 * TR], FP16, tag="t3", bufs=3)
        t3v = t3[:].rearrange("p (l r) -> p l r", l=3)
        nc.gpsimd.tensor_add(t3v, xv[:, :, 0], xv[:, :, 1])
        t2 = work.tile([128, TR], FP16, tag="t2", bufs=2)
        nc.gpsimd.tensor_add(t2[:], t3v[:, 0], t3v[:, 1])
        xs = work.tile([128, TR], FP16, tag="xs", bufs=3)
        # alternate the last tree add: DVE is the pacer, gpsimd has slack
        eng = nc.vector if ti % 2 == 0 else nc.gpsimd
        eng.tensor_add(xs[:], t2[:], t3v[:, 2])

        # ---- mm1 + exp ----
        exps = []
        for pi in range(3):
            ps_log = psum.tile([128, 1024], F32, tag="logits", bufs=2)
            for half in range(2):
                tp = 2 * pi + half
                nc.tensor.matmul(ps_log[:, half * 512:(half + 1) * 512],
                                 mw_sb[:, tp * 128:(tp + 1) * 128],
                                 xs[:], start=True, stop=True)
            ex = work.tile([128, 1024], BF16, tag="exp", bufs=8)
            nc.scalar.activation(ex[:], ps_log[:],
                                 mybir.ActivationFunctionType.Exp, bias=zbias[:])
            exps.append(ex)

        def expv(tp):
            return exps[tp // 2][:, (tp % 2) * 512:(tp % 2 + 1) * 512]

        # ---- mm2 + normalize per 128-row chunk ----
        vn = work.tile([128, NCH * T * D], FP16, tag="vn", bufs=3)
        for c in range(NCH):
            ps_val = psum.tile([128, 1024], F32, tag="val", bufs=2)
            for tp in range(6):
                off = 512 * (tp // 3) + 130 * (tp % 3)
                nc.tensor.matmul(ps_val[:, off:off + 130],
                                 expv(tp)[:, c * 128:(c + 1) * 128],
                                 mbd_sb[:, tp * 130:(tp + 1) * 130],
                                 start=True, stop=True)
            sums_ap = (ps_val[:].rearrange("p (h r) -> p h r", h=2)
                       [:, :, 0:390]
                       .rearrange("p h (a r) -> p h a r", a=3)
                       [:, :, :, 128:130])
            rec = work.tile([128, 12], F32, tag="rec", bufs=6)
            nc.vector.reciprocal(
                rec[:].rearrange("p (h a t) -> p h a t", h=2, a=3), sums_ap)
            in0 = (ps_val[:].rearrange("p (h r) -> p h r", h=2)
                   [:, :, 0:390]
                   .rearrange("p h (a r) -> p h a r", a=3)
                   [:, :, :, 0:128]
                   .rearrange("p h a (t d) -> p h a t d", t=2))
            in1 = (rec[:].rearrange("p (h a t) -> p h a t", h=2, a=3)
                   .unsqueeze(4)
                   .broadcast_to([128, 2, 3, 2, D]))
            outp = (vn[:, c * 768:(c + 1) * 768]
                    .rearrange("p (h a t d) -> p h a t d", h=2, a=3, t=2))
            nc.vector.tensor_mul(outp, in0, in1)
        # one fully-contiguous store per tile on the ACT HWDGE ring (keeps
        # the sync ring loads-only so prefetches issue unboundedly ahead);
        # 128 descriptors of 6 KB; host unshuffles [ti, p, c, t*d]
        nc.scalar.dma_start(out=out[ti], in_=vn[:])


_NC_CACHE = {}


def build_nc():
    if "nc" in _NC_CACHE:
        return _NC_CACHE["nc"]
    nc = bacc.Bacc("TRN2", target_bir_lowering=False, debug=False,
                   num_devices=NCORES)
    x_ap = nc.dram_tensor("x_sh", [NTILES, 128, 6 * TR], FP16,
                          kind="ExternalInput").ap()
    mw_ap = nc.dram_tensor("mw", [128, 6 * 128], FP16, kind="ExternalInput").ap()
    mbd_ap = nc.dram_tensor("mbd", [128, 6 * 130], FP16, kind="ExternalInput").ap()
    out_ap = nc.dram_tensor("out", [NTILES, 128, NCH * T * D], FP16,
                            kind="ExternalOutput").ap()
    with tile.TileContext(nc) as tc:
        kernel_body(tc, out_ap, x_ap, mw_ap, mbd_ap)
    nc.compile()
    _NC_CACHE["nc"] = nc
    return nc


def make_in_maps(x, M):
    xf = np.asarray(x).astype(F16)
    mw, mbd = build_consts(M)
    maps = []
    for i in range(NCORES):
        xc = xf[i * BS:(i + 1) * BS]                     # (8, 12, 883, 64)
        xc = xc.reshape(BS, 2, 6, N, D)                  # (b, lh, lr, n, d)
        xc = xc.transpose(0, 3, 1, 4, 2)                 # (b, n, lh, d, lr)
        xc = xc.reshape(ROWS, 2, D, 6)
        xp = np.zeros((RP, 2, D, 6), F16)
        xp[:ROWS] = xc
        xp = (xp.reshape(NTILES, TR, 128, 6)
                .transpose(0, 2, 3, 1)                   # (ti, p, lr, r)
                .reshape(NTILES, 128, 6 * TR))
        maps.append({"x_sh": np.ascontiguousarray(xp), "mw": mw, "mbd": mbd})
    return maps


def unshard_out(res):
    outs = []
    for i in range(NCORES):
        o = np.asarray(res[i]["out"]).astype(np.float32)   # [ti, p, c*768]
        o = (o.reshape(NTILES, 128, NCH, T * D)
              .transpose(0, 2, 1, 3)                       # row = ti*TR+c*128+p
              .reshape(RP, T * D))[:ROWS]
        outs.append(o.reshape(BS, N, T, D).transpose(0, 2, 1, 3))
    return np.ascontiguousarray(np.concatenate(outs, axis=0))


def kernel(x, M):
    nc = build_nc()
    in_maps = make_in_maps(x, M)
    res = run_bass_kernel_spmd(nc, in_maps, list(range(NCORES))).results
    return unshard_out(res)


if __name__ == "__main__":
    rng = np.random.default_rng(0)
    x = rng.standard_normal((B, L, N, D), dtype=np.float32)
    M = (rng.standard_normal((T, MNUM, D), dtype=np.float32) * 0.125).astype(np.float32)
    out = kernel(x, M)
    print("out", out.shape, out.dtype, float(np.abs(out).max()))
